# revision 17
# baseline (speedup 1.0000x reference)
"""MHA kernel for trn2: 8-core SPMD, core c = (batch c//2, head-group c%2 of 8 heads).

All-bf16, K=128-everywhere design (shapes hardcoded for B=4, S=2048,
HIDDEN=1024, H=16, DK=DV=64):
  - The PE streams the moving operand at half rate when the contraction K
    is 64 (measured 435ns vs 213ns for N=512), so qT/kT are stored with dk
    DUPLICATED across partitions ([dk;dk], 128 rows); scores psum = 2*qk
    with full-rate K=128 matmuls, exp scale 0.0625. The duplicate halves
    are filled by cheap SBUF->SBUF DMAs off the engines.
  - QKV projections in bf16, head-pair packed psums; Q/K biases folded
    into the evacuations (ACT Identity for K during idle phase A, DVE
    tensor_scalar for Q); V bias via a K=1 ones matmul.
  - mask multiply on DVE (7/8) and GpSimd (1/8) in bf16.
  - PV with bf16 ones-augmented V (row 64 = softmax denominator); deferred
    one score-tile (carry) so the PE never stalls on ACT's exp.
  - normalization straight from the pv psum: reciprocal of the den row,
    GpSimd partition_broadcast, DVE multiply -> oN bf16 (no oU staging).
  - out-projection drip-fed into the next block's score slots.
  Host sums the 2 group partials per batch + bo.
"""

import numpy as np
import ml_dtypes

import concourse.bacc as bacc
import concourse.mybir as mybir
import concourse.tile as tile
from concourse.bass_utils import run_bass_kernel_spmd

B, S, HID, H = 4, 2048, 1024, 16
DK = DV = 64
G = 2              # head groups per batch (8 heads each)
HPC, PAIRS = 8, 4  # heads / head-pairs per core
SQB = 512          # sq block
NJ = S // SQB      # 4
NT = S // 128      # 16 sk tiles
KTN = HID // 128   # 8 hidden k-tiles

F32 = mybir.dt.float32
F32R = mybir.dt.float32r
BF16 = mybir.dt.bfloat16
AF = mybir.ActivationFunctionType

EXP_SCALE = 0.0625   # scores psum carries 2*qk (dk duplicated), /8 softmax

_NC = None


def _build_nc():
    nc = bacc.Bacc("TRN2")
    xq_d = nc.declare_dram_parameter("xqT", [HID, S], BF16, isOutput=False)
    xk_d = nc.declare_dram_parameter("xkT", [HID, S], BF16, isOutput=False)
    xv_d = nc.declare_dram_parameter("xvT", [HID, S], BF16, isOutput=False)
    mk_d = nc.declare_dram_parameter("maskJ", [NJ, S, SQB], BF16, isOutput=False)
    wq_d = nc.declare_dram_parameter("wq", [HID, 512], BF16, isOutput=False)
    wk_d = nc.declare_dram_parameter("wk", [HID, 512], BF16, isOutput=False)
    wv_d = nc.declare_dram_parameter("wv", [HID, 512], BF16, isOutput=False)
    bq_d = nc.declare_dram_parameter("bqp", [128, 4], F32, isOutput=False)
    bk_d = nc.declare_dram_parameter("bkp", [128, 4], F32, isOutput=False)
    bv_d = nc.declare_dram_parameter("bv", [1, 512], BF16, isOutput=False)
    wo_d = nc.declare_dram_parameter("wo", [128, PAIRS, HID], BF16, isOutput=False)
    out_d = nc.declare_dram_parameter("out", [S, HID], F32, isOutput=True)

    with tile.TileContext(nc) as tc:
        with tc.tile_pool(name="persist", bufs=1) as PP, \
             tc.tile_pool(name="mskp", bufs=2) as MP, \
             tc.tile_pool(name="xp", bufs=2) as XP, \
             tc.tile_pool(name="ptp", bufs=3) as PTP, \
             tc.tile_pool(name="oup", bufs=2) as OUP, \
             tc.tile_pool(name="onp", bufs=2) as ONP, \
             tc.tile_pool(name="rtp", bufs=10) as RTP, \
             tc.tile_pool(name="bcp", bufs=2) as BCP, \
             tc.tile_pool(name="obp", bufs=2) as OBP:
            qT = PP.tile([128, HPC, S], BF16, name="qT")
            kT = PP.tile([128, HPC, S], BF16, name="kT")
            vA = PP.tile([128, NT, HPC, DV + 1], BF16, name="vA")
            wq_sb = PP.tile([128, KTN, 512], BF16, name="wq_sb")
            wk_sb = PP.tile([128, KTN, 512], BF16, name="wk_sb")
            wv_sb = PP.tile([128, KTN, 512], BF16, name="wv_sb")
            wo_sb = PP.tile([128, PAIRS, HID], BF16, name="wo_sb")
            bqp = PP.tile([128, 4], F32, name="bqp")
            bkp = PP.tile([128, 4], F32, name="bkp")
            bv_sb = PP.tile([1, 512], BF16, name="bv_sb")
            onesc = PP.tile([1, 128], BF16, name="onesc")

            # K's weights + bias first so its matmuls start ASAP; the rest
            # of the startup DMAs are emitted after the K-projection loop.
            nc.sync.dma_start(
                wk_sb[:], wk_d[:].rearrange("(k p) n -> p k n", p=128))
            nc.sync.dma_start(bkp[:], bk_d[:])
            nc.vector.memset(onesc[:], 1.0)
            nc.vector.memset(vA[:, :, :, DV:DV + 1], 1.0)

            def load_x(xd, n):
                x_sb = XP.tile([128, KTN, SQB], BF16, name="x_sb")
                nc.sync.dma_start(
                    x_sb[:],
                    xd[:, n * SQB:(n + 1) * SQB].rearrange(
                        "(k p) s -> p k s", p=128))
                return x_sb

            def qk_pair(x_sb, w_sb, dstT, b16, hp, n, on_act=False):
                # psum [128 = headA dk | headB dk, 512]
                ps_pool = qk_pair.pool
                ps = ps_pool.tile([128, SQB], F32, name="mp")
                for k in range(KTN):
                    nc.tensor.matmul(
                        ps[:], w_sb[:, k, hp * 128:(hp + 1) * 128],
                        x_sb[:, k, :], start=(k == 0), stop=(k == KTN - 1))
                sq = slice(n * SQB, (n + 1) * SQB)
                for r in range(2):
                    dst = dstT[0:64, 2 * hp + r, sq]
                    src = ps[64 * r:64 * r + 64, :]
                    bias = b16[64 * r:64 * r + 64, hp:hp + 1]
                    if on_act:
                        nc.scalar.activation(dst, src, AF.Identity,
                                             bias=bias, scale=1.0)
                    else:
                        nc.vector.tensor_scalar(
                            dst, src, 1.0, bias,
                            mybir.AluOpType.mult, mybir.AluOpType.add)
                # duplicate dk rows 0:64 -> 64:128 so scores run with K=128
                nc.sync.dma_start(dstT[64:128, 2 * hp:2 * hp + 2, sq],
                                  dstT[0:64, 2 * hp:2 * hp + 2, sq])

            # ---------------- Phase A: K, V, Q(block 0) projections ----------
            with tc.tile_pool(name="pra", bufs=2, space="PSUM") as PRA:
                qk_pair.pool = PRA
                # K (all blocks), evac on ACT (idle until exp stream starts)
                for n in range(NJ):
                    x_sb = load_x(xk_d, n)
                    for hp in range(PAIRS):
                        qk_pair(x_sb, wk_sb, kT, bkp, hp, n, on_act=True)
                # deferred startup DMAs (overlap with the K projections)
                nc.sync.dma_start(
                    wv_sb[:], wv_d[:].rearrange("(k p) n -> p k n", p=128))
                nc.sync.dma_start(bv_sb[:], bv_d[:])
                nc.sync.dma_start(
                    wq_sb[:], wq_d[:].rearrange("(k p) n -> p k n", p=128))
                nc.sync.dma_start(bqp[:], bq_d[:])
                # V (all blocks), bias via K=1 matmul, DVE evac
                msk0 = None
                for n in range(NJ):
                    x_sb = load_x(xv_d, n)
                    if n == 1:
                        msk0 = MP.tile([128, NT, SQB], BF16, name="msk")
                        nc.sync.dma_start(
                            msk0[:],
                            mk_d[0].rearrange("(t p) s -> p t s", p=128))
                    if n == 2:
                        nc.sync.dma_start(wo_sb[:], wo_d[:])
                    for stl in range(4):
                        st = n * 4 + stl
                        ps = PRA.tile([128, HPC, DV], F32, name="ps_v")
                        for k in range(KTN):
                            nc.tensor.matmul(
                                ps[:], x_sb[:, k, stl * 128:(stl + 1) * 128],
                                wv_sb[:, k, :], start=(k == 0), stop=False)
                        nc.tensor.matmul(ps[:], onesc[:], bv_sb[:],
                                         start=False, stop=True)
                        nc.vector.tensor_copy(vA[:, st, :, 0:DV], ps[:])
                # Q block 0, DVE evac
                xq_sb = load_x(xq_d, 0)
                for hp in range(PAIRS):
                    qk_pair(xq_sb, wq_sb, qT, bqp, hp, 0)

            # ---------------- Phase B: attention + out-projection ------------
            with tc.tile_pool(name="misc", bufs=2, space="PSUM") as MISC, \
                 tc.tile_pool(name="scps", bufs=2, space="PSUM") as SCP, \
                 tc.tile_pool(name="pvps", bufs=2, space="PSUM") as PVP:

                def tail_steps(j, oU, oN, rcs):
                    steps = []
                    for hl in range(HPC):
                        hp, r = divmod(hl, 2)
                        pb = 64 * r

                        def s_bc(hl=hl, hp=hp, pb=pb):
                            bc = BCP.tile([128, SQB], BF16, name="bc")
                            nc.sync.dma_start(
                                bc[:], rcs[hl][0:1, None, :].broadcast_to(
                                    [1, 128, SQB]))
                            nc.vector.tensor_mul(oN[pb:pb + DV, hp, :],
                                                 oU[pb:pb + DV, hp, :],
                                                 bc[pb:pb + DV, :])
                        steps.append(s_bc)
                    opt = {}
                    for stl in range(4):
                        for nn in range(2):
                            def s_op_a(stl=stl, nn=nn):
                                op = MISC.tile([128, SQB], F32, name="mp")
                                opt['t'] = op
                                for hp in range(2):
                                    nc.tensor.matmul(
                                        op[:],
                                        oN[:, hp, stl * 128:(stl + 1) * 128],
                                        wo_sb[:, hp, nn * SQB:(nn + 1) * SQB],
                                        start=(hp == 0), stop=False)

                            def s_op_b(stl=stl, nn=nn, j=j):
                                st = 4 * j + stl
                                op = opt['t']
                                for hp in range(2, PAIRS):
                                    nc.tensor.matmul(
                                        op[:],
                                        oN[:, hp, stl * 128:(stl + 1) * 128],
                                        wo_sb[:, hp, nn * SQB:(nn + 1) * SQB],
                                        start=False, stop=(hp == PAIRS - 1))
                                ob = OBP.tile([128, SQB], F32, name="ob")
                                nc.vector.tensor_copy(ob[:], op[:])
                                nc.sync.dma_start(
                                    out_d[st * 128:(st + 1) * 128,
                                          nn * SQB:(nn + 1) * SQB], ob[:])
                            steps.append(s_op_a)
                            steps.append(s_op_b)
                    return steps

                def q_steps(n):
                    steps = []
                    qs = {}

                    def s_load():
                        q_steps.x_sb = load_x(xq_d, n)
                    steps.append(s_load)
                    for hp in range(PAIRS):
                        def s_h1(hp=hp):
                            ps = MISC.tile([128, SQB], F32, name="mp")
                            qs['t'] = ps
                            for k in range(4):
                                nc.tensor.matmul(
                                    ps[:],
                                    wq_sb[:, k, hp * 128:(hp + 1) * 128],
                                    q_steps.x_sb[:, k, :], start=(k == 0),
                                    stop=False)

                        def s_h2(hp=hp, n=n):
                            ps = qs['t']
                            for k in range(4, KTN):
                                nc.tensor.matmul(
                                    ps[:],
                                    wq_sb[:, k, hp * 128:(hp + 1) * 128],
                                    q_steps.x_sb[:, k, :], start=False,
                                    stop=(k == KTN - 1))
                            sq = slice(n * SQB, (n + 1) * SQB)
                            for r in range(2):
                                nc.vector.tensor_scalar(
                                    qT[0:64, 2 * hp + r, sq],
                                    ps[64 * r:64 * r + 64, :],
                                    1.0, bqp[64 * r:64 * r + 64, hp:hp + 1],
                                    mybir.AluOpType.mult,
                                    mybir.AluOpType.add)
                            nc.sync.dma_start(
                                qT[64:128, 2 * hp:2 * hp + 2, sq],
                                qT[0:64, 2 * hp:2 * hp + 2, sq])
                        steps.append(s_h1)
                        steps.append(s_h2)
                    return steps

                def emit_pv(c):
                    # PV matmuls for the previous score tile-pair; deferred one
                    # group so the PE never stalls waiting on ACT's exp.
                    cpv, cpt, ctt, chl, cpb, chp, coU, crcs, fin = c
                    for u in range(2):
                        nc.tensor.matmul(
                            cpv[:], vA[:, 2 * ctt + u, chl, :], cpt[:, u, :],
                            start=(ctt == 0 and u == 0), stop=(fin and u == 1))
                    if not fin:
                        return
                    # off-PE: 1/den row + unnormalized out to SBUF
                    rct = RTP.tile([1, SQB], BF16, name="rct")
                    with nc.allow_low_precision(reason="bf16 recip"):
                        nc.vector.reciprocal(rct[:], cpv[DV:DV + 1, :])
                    crcs.append(rct)
                    nc.vector.tensor_copy(coU[cpb:cpb + DV, chp, :],
                                          cpv[0:DV, :])

                carry = None
                qpend = q_steps(1)
                steps = []
                for j in range(NJ):
                    if j == 0:
                        msk = msk0
                    else:
                        msk = MP.tile([128, NT, SQB], BF16, name="msk")
                        nc.sync.dma_start(
                            msk[:], mk_d[j].rearrange("(t p) s -> p t s", p=128))
                    oU = OUP.tile([128, PAIRS, SQB], BF16, name="oU")
                    oN = ONP.tile([128, PAIRS, SQB], BF16, name="oN")
                    rcs = []
                    for hl in range(HPC):
                        hp, r = divmod(hl, 2)
                        pb = 64 * r
                        pv = PVP.tile([DV + 1, SQB], F32, name="pv")
                        for tt in range(NT // 2):
                            sc = SCP.tile([128, 2, SQB], F32, name="sc")
                            for u in range(2):
                                t = 2 * tt + u
                                nc.tensor.matmul(
                                    sc[:, u, :],
                                    kT[:, hl, t * 128:(t + 1) * 128],
                                    qT[:, hl, j * SQB:(j + 1) * SQB],
                                    start=True, stop=True)
                            if carry is not None:
                                emit_pv(carry)
                            pt = PTP.tile([128, 2, SQB], BF16, name="pt")
                            nc.scalar.activation(pt[:], sc[:], AF.Exp,
                                                 scale=EXP_SCALE)
                            nc.vector.tensor_mul(pt[:], pt[:],
                                                 msk[:, 2 * tt:2 * tt + 2, :])
                            carry = (pv, pt, tt, hl, pb, hp, oU, rcs,
                                     tt == NT // 2 - 1)
                            # drip-feed previous block's tail + next Q block
                            gi = hl * (NT // 2) + tt
                            if steps and gi >= 2 and gi % 2 == 0:
                                steps.pop(0)()
                            if qpend and gi % 6 == 3:
                                qpend.pop(0)()
                    while steps:
                        steps.pop(0)()
                    while qpend:
                        qpend.pop(0)()
                    steps = tail_steps(j, oU, oN, rcs)
                    if j < NJ - 2:
                        qpend = q_steps(j + 2)
                # final PV + final block's tail
                emit_pv(carry)
                for s in steps:
                    s()
    nc.finalize()
    return nc


def get_nc():
    global _NC
    if _NC is None:
        _NC = _build_nc()
    return _NC


def make_in_maps(q_hidden_inputs, k_hidden_inputs, v_hidden_inputs, mask,
                 wq, bq, wk, bk, wv, bv, wo, bo):
    f32 = np.float32
    bf16 = ml_dtypes.bfloat16
    in_maps = []
    per_batch = []
    for b in range(B):
        xqT = np.asarray(q_hidden_inputs[b]).T.astype(bf16)
        xkT = np.asarray(k_hidden_inputs[b]).T.astype(bf16)
        xvT = np.asarray(v_hidden_inputs[b]).T.astype(bf16)
        maskT = np.asarray(mask[b]).T.astype(bf16)             # [sk, sq]
        maskJ = np.ascontiguousarray(
            maskT.reshape(S, NJ, SQB).transpose(1, 0, 2))     # [j, sk, 512]
        per_batch.append((xqT, xkT, xvT, maskJ))

    def pack_b(bias, hs):
        # [8, 64] -> [128, 4]: b[r*64+dk, hp] = bias[2hp+r, dk]
        bb = np.asarray(bias[hs], dtype=f32).reshape(PAIRS, 2 * DK).T
        return np.ascontiguousarray(bb)

    for c in range(2 * B):
        b, g = divmod(c, 2)
        xqT, xkT, xvT, maskJ = per_batch[b]
        hs = slice(g * HPC, (g + 1) * HPC)
        in_maps.append({
            "xqT": xqT, "xkT": xkT, "xvT": xvT, "maskJ": maskJ,
            "wq": np.ascontiguousarray(
                np.asarray(wq[hs], dtype=f32).transpose(1, 0, 2)
                .reshape(HID, 512)).astype(bf16),
            "wk": np.ascontiguousarray(
                np.asarray(wk[hs], dtype=f32).transpose(1, 0, 2)
                .reshape(HID, 512)).astype(bf16),
            "wv": np.ascontiguousarray(
                np.asarray(wv[hs], dtype=f32).transpose(1, 0, 2)
                .reshape(HID, 512)).astype(bf16),
            "bqp": pack_b(bq, hs),
            "bkp": pack_b(bk, hs),
            "bv": np.ascontiguousarray(
                np.asarray(bv[hs], dtype=f32).reshape(1, 512)).astype(bf16),
            "wo": np.ascontiguousarray(
                np.asarray(wo[g * 512:(g + 1) * 512, :], dtype=f32)
                .reshape(PAIRS, 128, HID).transpose(1, 0, 2)).astype(bf16),
        })
    return in_maps


def assemble(results, bo):
    out = np.empty((B, S, HID), dtype=np.float32)
    for b in range(B):
        out[b] = results[2 * b]["out"] + results[2 * b + 1]["out"] \
            + bo.astype(np.float32)[None, :]
    return out


def run(inputs, trace=False, **kw):
    nc = get_nc()
    in_maps = make_in_maps(**inputs)
    bkr = run_bass_kernel_spmd(nc, in_maps, list(range(2 * B)), trace=trace, **kw)
    return assemble(bkr.results, np.asarray(inputs["bo"])), bkr


def kernel(**inputs):
    out, _ = run(inputs, trace=False)
    return out


# revision 18
# speedup vs baseline: 1.1505x; 1.1505x over previous
"""MHA kernel for trn2: 8-core SPMD, core c = (batch c//2, head-group c%2 of 8 heads).

All-bf16, K=128-everywhere design (shapes hardcoded for B=4, S=2048,
HIDDEN=1024, H=16, DK=DV=64):
  - The PE streams the moving operand at half rate when the contraction K
    is 64 (measured 435ns vs 213ns for N=512), so qT/kT are stored with dk
    DUPLICATED across partitions ([dk;dk], 128 rows); scores psum = 2*qk
    with full-rate K=128 matmuls, exp scale 0.0625. The duplicate halves
    are filled by cheap SBUF->SBUF DMAs off the engines.
  - QKV projections in bf16, head-pair packed psums; Q/K biases folded
    into the evacuations (ACT Identity for K during idle phase A, DVE
    tensor_scalar for Q); V bias via a K=1 ones matmul.
  - mask multiply on DVE (7/8) and GpSimd (1/8) in bf16.
  - PV with bf16 ones-augmented V (row 64 = softmax denominator); deferred
    one score-tile (carry) so the PE never stalls on ACT's exp.
  - normalization straight from the pv psum: reciprocal of the den row,
    GpSimd partition_broadcast, DVE multiply -> oN bf16 (no oU staging).
  - out-projection drip-fed into the next block's score slots.
  Host sums the 2 group partials per batch + bo.
"""

import numpy as np
import ml_dtypes

import concourse.bacc as bacc
import concourse.mybir as mybir
import concourse.tile as tile
from concourse.bass_utils import run_bass_kernel_spmd

B, S, HID, H = 4, 2048, 1024, 16
DK = DV = 64
G = 2              # head groups per batch (8 heads each)
HPC, PAIRS = 8, 4  # heads / head-pairs per core
SQB = 512          # sq block
NJ = S // SQB      # 4
NT = S // 128      # 16 sk tiles
KTN = HID // 128   # 8 hidden k-tiles

F32 = mybir.dt.float32
F32R = mybir.dt.float32r
BF16 = mybir.dt.bfloat16
AF = mybir.ActivationFunctionType

EXP_SCALE = 0.0625   # scores psum carries 2*qk (dk duplicated), /8 softmax

_NC = None


def _build_nc():
    nc = bacc.Bacc("TRN2")
    xq_d = nc.declare_dram_parameter("xqT", [HID, S], BF16, isOutput=False)
    xk_d = nc.declare_dram_parameter("xkT", [HID, S], BF16, isOutput=False)
    xv_d = nc.declare_dram_parameter("xvT", [HID, S], BF16, isOutput=False)
    mk_d = nc.declare_dram_parameter("maskJ", [NJ, S, SQB], BF16, isOutput=False)
    wq_d = nc.declare_dram_parameter("wq", [HID, 512], BF16, isOutput=False)
    wk_d = nc.declare_dram_parameter("wk", [HID, 512], BF16, isOutput=False)
    wv_d = nc.declare_dram_parameter("wv", [HID, 512], BF16, isOutput=False)
    bq_d = nc.declare_dram_parameter("bqp", [128, 4], F32, isOutput=False)
    bk_d = nc.declare_dram_parameter("bkp", [128, 4], F32, isOutput=False)
    bv_d = nc.declare_dram_parameter("bv", [1, 512], BF16, isOutput=False)
    wo_d = nc.declare_dram_parameter("wo", [128, PAIRS, HID], BF16, isOutput=False)
    out_d = nc.declare_dram_parameter("out", [S, HID], F32, isOutput=True)

    with tile.TileContext(nc) as tc:
        with tc.tile_pool(name="persist", bufs=1) as PP, \
             tc.tile_pool(name="mskp", bufs=2) as MP, \
             tc.tile_pool(name="xp", bufs=2) as XP, \
             tc.tile_pool(name="ptp", bufs=3) as PTP, \
             tc.tile_pool(name="oup", bufs=2) as OUP, \
             tc.tile_pool(name="onp", bufs=2) as ONP, \
             tc.tile_pool(name="rtp", bufs=10) as RTP, \
             tc.tile_pool(name="obp", bufs=2) as OBP:
            qT = PP.tile([128, HPC, S], BF16, name="qT")
            kT = PP.tile([128, HPC, S], BF16, name="kT")
            vA = PP.tile([128, NT, HPC, DV + 1], BF16, name="vA")
            wq_sb = PP.tile([128, KTN, 512], BF16, name="wq_sb")
            wk_sb = PP.tile([128, KTN, 512], BF16, name="wk_sb")
            wv_sb = PP.tile([128, KTN, 512], BF16, name="wv_sb")
            wo_sb = PP.tile([128, PAIRS, HID], BF16, name="wo_sb")
            bqp = PP.tile([128, 4], F32, name="bqp")
            bkp = PP.tile([128, 4], F32, name="bkp")
            bv_sb = PP.tile([1, 512], BF16, name="bv_sb")
            onesc = PP.tile([1, 128], BF16, name="onesc")

            # K's weights + bias first so its matmuls start ASAP; the rest
            # of the startup DMAs are emitted after the K-projection loop.
            nc.sync.dma_start(
                wk_sb[:], wk_d[:].rearrange("(k p) n -> p k n", p=128))
            nc.sync.dma_start(bkp[:], bk_d[:])
            nc.vector.memset(onesc[:], 1.0)
            nc.vector.memset(vA[:, :, :, DV:DV + 1], 1.0)

            def load_x(xd, n):
                x_sb = XP.tile([128, KTN, SQB], BF16, name="x_sb")
                nc.sync.dma_start(
                    x_sb[:],
                    xd[:, n * SQB:(n + 1) * SQB].rearrange(
                        "(k p) s -> p k s", p=128))
                return x_sb

            def qk_pair(x_sb, w_sb, dstT, b16, hp, n, on_act=False):
                # psum [128 = headA dk | headB dk, 512]
                ps_pool = qk_pair.pool
                ps = ps_pool.tile([128, SQB], F32, name="mp")
                for k in range(KTN):
                    nc.tensor.matmul(
                        ps[:], w_sb[:, k, hp * 128:(hp + 1) * 128],
                        x_sb[:, k, :], start=(k == 0), stop=(k == KTN - 1))
                sq = slice(n * SQB, (n + 1) * SQB)
                for r in range(2):
                    dst = dstT[0:64, 2 * hp + r, sq]
                    src = ps[64 * r:64 * r + 64, :]
                    bias = b16[64 * r:64 * r + 64, hp:hp + 1]
                    if on_act:
                        nc.scalar.activation(dst, src, AF.Identity,
                                             bias=bias, scale=1.0)
                    else:
                        nc.vector.tensor_scalar(
                            dst, src, 1.0, bias,
                            mybir.AluOpType.mult, mybir.AluOpType.add)
                # duplicate dk rows 0:64 -> 64:128 so scores run with K=128
                nc.sync.dma_start(dstT[64:128, 2 * hp:2 * hp + 2, sq],
                                  dstT[0:64, 2 * hp:2 * hp + 2, sq])

            # ---------------- Phase A: K, V, Q(block 0) projections ----------
            with tc.tile_pool(name="pra", bufs=2, space="PSUM") as PRA:
                qk_pair.pool = PRA
                # K (all blocks), evac on ACT (idle until exp stream starts)
                for n in range(NJ):
                    x_sb = load_x(xk_d, n)
                    for hp in range(PAIRS):
                        qk_pair(x_sb, wk_sb, kT, bkp, hp, n, on_act=True)
                # deferred startup DMAs (overlap with the K projections)
                nc.sync.dma_start(
                    wv_sb[:], wv_d[:].rearrange("(k p) n -> p k n", p=128))
                nc.sync.dma_start(bv_sb[:], bv_d[:])
                nc.sync.dma_start(
                    wq_sb[:], wq_d[:].rearrange("(k p) n -> p k n", p=128))
                nc.sync.dma_start(bqp[:], bq_d[:])
                # V (all blocks), bias via K=1 matmul, DVE evac
                msk0 = None
                for n in range(NJ):
                    x_sb = load_x(xv_d, n)
                    if n == 1:
                        msk0 = MP.tile([128, NT, SQB], BF16, name="msk")
                        nc.sync.dma_start(
                            msk0[:],
                            mk_d[0].rearrange("(t p) s -> p t s", p=128))
                    if n == 2:
                        nc.sync.dma_start(wo_sb[:], wo_d[:])
                    for stl in range(4):
                        st = n * 4 + stl
                        ps = PRA.tile([128, HPC, DV], F32, name="ps_v")
                        for k in range(KTN):
                            nc.tensor.matmul(
                                ps[:], x_sb[:, k, stl * 128:(stl + 1) * 128],
                                wv_sb[:, k, :], start=(k == 0), stop=False)
                        nc.tensor.matmul(ps[:], onesc[:], bv_sb[:],
                                         start=False, stop=True)
                        nc.vector.tensor_copy(vA[:, st, :, 0:DV], ps[:])
                # Q block 0, DVE evac
                xq_sb = load_x(xq_d, 0)
                for hp in range(PAIRS):
                    qk_pair(xq_sb, wq_sb, qT, bqp, hp, 0)

            # ---------------- Phase B: attention + out-projection ------------
            with tc.tile_pool(name="misc", bufs=2, space="PSUM") as MISC, \
                 tc.tile_pool(name="scps", bufs=2, space="PSUM") as SCP, \
                 tc.tile_pool(name="pvps", bufs=2, space="PSUM") as PVP:

                def tail_steps(j, oU, oN, rcs):
                    steps = []
                    for hl in range(HPC):
                        hp, r = divmod(hl, 2)
                        pb = 64 * r

                        def s_bc(hl=hl, hp=hp, pb=pb):
                            bc = MISC.tile([128, SQB], F32, name="mp")
                            nc.tensor.matmul(bc[0:DV, :], onesc[0:1, 0:DV],
                                             rcs[hl][:], start=True, stop=True)
                            nc.vector.tensor_mul(oN[pb:pb + DV, hp, :],
                                                 oU[pb:pb + DV, hp, :],
                                                 bc[0:DV, :])
                        steps.append(s_bc)
                    opt = {}
                    for stl in range(4):
                        for nn in range(2):
                            def s_op_a(stl=stl, nn=nn):
                                op = MISC.tile([128, SQB], F32, name="mp")
                                opt['t'] = op
                                for hp in range(2):
                                    nc.tensor.matmul(
                                        op[:],
                                        oN[:, hp, stl * 128:(stl + 1) * 128],
                                        wo_sb[:, hp, nn * SQB:(nn + 1) * SQB],
                                        start=(hp == 0), stop=False)

                            def s_op_b(stl=stl, nn=nn, j=j):
                                st = 4 * j + stl
                                op = opt['t']
                                for hp in range(2, PAIRS):
                                    nc.tensor.matmul(
                                        op[:],
                                        oN[:, hp, stl * 128:(stl + 1) * 128],
                                        wo_sb[:, hp, nn * SQB:(nn + 1) * SQB],
                                        start=False, stop=(hp == PAIRS - 1))
                                ob = OBP.tile([128, SQB], F32, name="ob")
                                nc.vector.tensor_copy(ob[:], op[:])
                                nc.sync.dma_start(
                                    out_d[st * 128:(st + 1) * 128,
                                          nn * SQB:(nn + 1) * SQB], ob[:])
                            steps.append(s_op_a)
                            steps.append(s_op_b)
                    return steps

                def q_steps(n):
                    steps = []
                    qs = {}

                    def s_load():
                        q_steps.x_sb = load_x(xq_d, n)
                    steps.append(s_load)
                    for hp in range(PAIRS):
                        def s_h1(hp=hp):
                            ps = MISC.tile([128, SQB], F32, name="mp")
                            qs['t'] = ps
                            for k in range(4):
                                nc.tensor.matmul(
                                    ps[:],
                                    wq_sb[:, k, hp * 128:(hp + 1) * 128],
                                    q_steps.x_sb[:, k, :], start=(k == 0),
                                    stop=False)

                        def s_h2(hp=hp, n=n):
                            ps = qs['t']
                            for k in range(4, KTN):
                                nc.tensor.matmul(
                                    ps[:],
                                    wq_sb[:, k, hp * 128:(hp + 1) * 128],
                                    q_steps.x_sb[:, k, :], start=False,
                                    stop=(k == KTN - 1))
                            sq = slice(n * SQB, (n + 1) * SQB)
                            for r in range(2):
                                nc.vector.tensor_scalar(
                                    qT[0:64, 2 * hp + r, sq],
                                    ps[64 * r:64 * r + 64, :],
                                    1.0, bqp[64 * r:64 * r + 64, hp:hp + 1],
                                    mybir.AluOpType.mult,
                                    mybir.AluOpType.add)
                            nc.sync.dma_start(
                                qT[64:128, 2 * hp:2 * hp + 2, sq],
                                qT[0:64, 2 * hp:2 * hp + 2, sq])
                        steps.append(s_h1)
                        steps.append(s_h2)
                    return steps

                def emit_pv(c):
                    # PV matmuls for the previous score tile-pair; deferred one
                    # group so the PE never stalls waiting on ACT's exp.
                    cpv, cpt, ctt, chl, cpb, chp, coU, crcs, fin = c
                    for u in range(2):
                        nc.tensor.matmul(
                            cpv[:], vA[:, 2 * ctt + u, chl, :], cpt[:, u, :],
                            start=(ctt == 0 and u == 0), stop=(fin and u == 1))
                    if not fin:
                        return
                    # off-PE: 1/den row + unnormalized out to SBUF
                    rct = RTP.tile([1, SQB], BF16, name="rct")
                    with nc.allow_low_precision(reason="bf16 recip"):
                        nc.vector.reciprocal(rct[:], cpv[DV:DV + 1, :])
                    crcs.append(rct)
                    nc.vector.tensor_copy(coU[cpb:cpb + DV, chp, :],
                                          cpv[0:DV, :])

                carry = None
                qpend = q_steps(1)
                steps = []
                for j in range(NJ):
                    if j == 0:
                        msk = msk0
                    else:
                        msk = MP.tile([128, NT, SQB], BF16, name="msk")
                        nc.sync.dma_start(
                            msk[:], mk_d[j].rearrange("(t p) s -> p t s", p=128))
                    oU = OUP.tile([128, PAIRS, SQB], BF16, name="oU")
                    oN = ONP.tile([128, PAIRS, SQB], BF16, name="oN")
                    rcs = []
                    for hl in range(HPC):
                        hp, r = divmod(hl, 2)
                        pb = 64 * r
                        pv = PVP.tile([DV + 1, SQB], F32, name="pv")
                        for tt in range(NT // 2):
                            sc = SCP.tile([128, 2, SQB], F32, name="sc")
                            for u in range(2):
                                t = 2 * tt + u
                                nc.tensor.matmul(
                                    sc[:, u, :],
                                    kT[:, hl, t * 128:(t + 1) * 128],
                                    qT[:, hl, j * SQB:(j + 1) * SQB],
                                    start=True, stop=True)
                            if carry is not None:
                                emit_pv(carry)
                            pt = PTP.tile([128, 2, SQB], BF16, name="pt")
                            nc.scalar.activation(pt[:], sc[:], AF.Exp,
                                                 scale=EXP_SCALE)
                            nc.vector.tensor_mul(pt[:], pt[:],
                                                 msk[:, 2 * tt:2 * tt + 2, :])
                            carry = (pv, pt, tt, hl, pb, hp, oU, rcs,
                                     tt == NT // 2 - 1)
                            # drip-feed previous block's tail + next Q block
                            gi = hl * (NT // 2) + tt
                            if steps and gi >= 2 and gi % 2 == 0:
                                steps.pop(0)()
                            if qpend and gi % 6 == 3:
                                qpend.pop(0)()
                    while steps:
                        steps.pop(0)()
                    while qpend:
                        qpend.pop(0)()
                    steps = tail_steps(j, oU, oN, rcs)
                    if j < NJ - 2:
                        qpend = q_steps(j + 2)
                # final PV + final block's tail
                emit_pv(carry)
                for s in steps:
                    s()
    nc.finalize()
    return nc


def get_nc():
    global _NC
    if _NC is None:
        _NC = _build_nc()
    return _NC


def make_in_maps(q_hidden_inputs, k_hidden_inputs, v_hidden_inputs, mask,
                 wq, bq, wk, bk, wv, bv, wo, bo):
    f32 = np.float32
    bf16 = ml_dtypes.bfloat16
    in_maps = []
    per_batch = []
    for b in range(B):
        xqT = np.asarray(q_hidden_inputs[b]).T.astype(bf16)
        xkT = np.asarray(k_hidden_inputs[b]).T.astype(bf16)
        xvT = np.asarray(v_hidden_inputs[b]).T.astype(bf16)
        maskT = np.asarray(mask[b]).T.astype(bf16)             # [sk, sq]
        maskJ = np.ascontiguousarray(
            maskT.reshape(S, NJ, SQB).transpose(1, 0, 2))     # [j, sk, 512]
        per_batch.append((xqT, xkT, xvT, maskJ))

    def pack_b(bias, hs):
        # [8, 64] -> [128, 4]: b[r*64+dk, hp] = bias[2hp+r, dk]
        bb = np.asarray(bias[hs], dtype=f32).reshape(PAIRS, 2 * DK).T
        return np.ascontiguousarray(bb)

    for c in range(2 * B):
        b, g = divmod(c, 2)
        xqT, xkT, xvT, maskJ = per_batch[b]
        hs = slice(g * HPC, (g + 1) * HPC)
        in_maps.append({
            "xqT": xqT, "xkT": xkT, "xvT": xvT, "maskJ": maskJ,
            "wq": np.ascontiguousarray(
                np.asarray(wq[hs], dtype=f32).transpose(1, 0, 2)
                .reshape(HID, 512)).astype(bf16),
            "wk": np.ascontiguousarray(
                np.asarray(wk[hs], dtype=f32).transpose(1, 0, 2)
                .reshape(HID, 512)).astype(bf16),
            "wv": np.ascontiguousarray(
                np.asarray(wv[hs], dtype=f32).transpose(1, 0, 2)
                .reshape(HID, 512)).astype(bf16),
            "bqp": pack_b(bq, hs),
            "bkp": pack_b(bk, hs),
            "bv": np.ascontiguousarray(
                np.asarray(bv[hs], dtype=f32).reshape(1, 512)).astype(bf16),
            "wo": np.ascontiguousarray(
                np.asarray(wo[g * 512:(g + 1) * 512, :], dtype=f32)
                .reshape(PAIRS, 128, HID).transpose(1, 0, 2)).astype(bf16),
        })
    return in_maps


def assemble(results, bo):
    out = np.empty((B, S, HID), dtype=np.float32)
    for b in range(B):
        out[b] = results[2 * b]["out"] + results[2 * b + 1]["out"] \
            + bo.astype(np.float32)[None, :]
    return out


def run(inputs, trace=False, **kw):
    nc = get_nc()
    in_maps = make_in_maps(**inputs)
    bkr = run_bass_kernel_spmd(nc, in_maps, list(range(2 * B)), trace=trace, **kw)
    return assemble(bkr.results, np.asarray(inputs["bo"])), bkr


def kernel(**inputs):
    out, _ = run(inputs, trace=False)
    return out
